# revision 14
# baseline (speedup 1.0000x reference)
"""Trainium2 Bass kernel for DFMN prototypical-network loss (retrieval_knn).

Reference math (per pixel, 64-way episode):
    protos = prototypes[indices]                         # [64, C]
    logits = -(|q|^2 + |p|^2 - 2 q.p)                    # [N, 64]
    loss   = -mean(log_softmax(logits)[label])

Key identity: the per-pixel |q|^2 term is constant across classes, so it
cancels in log_softmax.  With G = q.p and z = 2G - pn (pn = |p|^2 per class):
    -logp[label] = LSE_c(z) - z[label]
    loss = [ sum_px LSE_c(z) - sum_img (2*rowsumG[label_b] - 196*pn[label_b]) ] / N

Device layout per core (64 images, data-parallel over 8 cores).  Work is
organized in 16 "units" of 2 image pairs; the two pairs of a unit occupy the
two partition halves of one PSUM bank via PE column tiling (tile_position
(0,0) / (0,64)), so the exp and reductions run on full 128-partition tiles:
  - G via TensorE:   G[128, 392] = protosT_k.T @ Q_k  (8 K-chunks, fp16,
    col-tiled pair of matmuls per unit; prototype weights loaded once per
    K-chunk per 3-unit group and redundant LDWEIGHTS deduplicated)
  - exp via ScalarE: e = Exp(2*G - pn)  (PSUM -> SBUF bf16, fused scale/bias)
  - colsum via TensorE: s[32, 392] += selector.T @ e  (selector has a ones
    column per partition half; accumulated across all 16 units in one bank)
  - label rowsums via VectorE: r2[128, 32] = per-image free-dim reduce of G
  - final ScalarE Ln with accum_out -> per-row sums of log s
Host finishes: label gather from r2, pn terms, exact float64 mean.

Streaming dtype is fp16: halves the HBM traffic (the roofline bottleneck:
~25.7 MB/core at ~358 GB/s) and runs the PE at 1 cycle/row (fp32 is 4).
"""

import sys

for _p in ("/opt/trn_rl_repo",):
    if _p not in sys.path:
        sys.path.insert(0, _p)

import numpy as np

import concourse.bass as bass
import concourse.bacc as bacc
import concourse.tile as tile
from concourse import mybir
from concourse.bass_utils import run_bass_kernel_spmd

# Problem constants (nn_DFMNLoss: B=512, C=1024, 14x14 features, 64-way)
B = 512
C = 1024
F2 = 196          # 14 * 14 pixels per image
NWAY = 64
NCORES = 8
BPC = B // NCORES           # 64 images per core
NPAIR = BPC // 2            # 32 image pairs per core
NU = NPAIR // 2             # 16 col-tiled units (2 pairs each)
UPG = 3                     # units per group (PSUM: 2*UPG g-banks + 1 s-bank)
KT = C // 128               # 8 contraction chunks of 128 channels
PAIRCOLS = 2 * F2           # 392 pixel columns per pair
QCOLS = KT * PAIRCOLS       # 6272 fp16 per partition per pair

F32 = mybir.dt.float32
F16 = mybir.dt.float16
F8 = mybir.dt.float8e4
F8_NP = mybir.dt.np(F8)
BF16 = mybir.dt.bfloat16
BF16_NP = mybir.dt.np(BF16)

_CACHE = {}


def _dedup_ldweights(nc):
    """Drop InstLdweights that reload weights already resident in the PE
    array.  Tile emits one LDWEIGHTS per matmul; inside a group the same
    prototype chunk is loaded for every unit, and the loads serialize with
    the matmul stream on the PE.  Matmults here are non-self-loading
    (ldweights=False), so a dropped reload just reuses the array contents.
    Tracks state per 32-wide PE column group; only sync-free LDWs are
    dropped, and any dangling dependency names are remapped to the keeper.
    """
    removed = {}
    for blk in nc.m.functions[0].blocks:
        state = {}  # col_group -> (key, keeper_name)
        kept = []
        for inst in blk.instructions:
            if isinstance(inst, mybir.InstLdweights):
                a = inst.ins[0]
                tp = inst.tile_position or (0, 0)
                ts = inst.tile_size or (128, 128)
                key = (a.memref, a.offset, str(a.ap), str(a.dtype), tp, ts)
                cgs = range(tp[1] // 32, (tp[1] + ts[1] + 31) // 32)
                si = inst.sync_info
                clean = si is None or (not si.on_wait and not si.on_update)
                prev = [state.get(cg) for cg in cgs]
                if clean and all(p is not None and p[0] == key for p in prev):
                    removed[inst.name] = prev[0][1]
                    continue
                for cg in cgs:
                    state[cg] = (key, inst.name)
            kept.append(inst)
        blk.instructions[:] = kept
    if removed:
        for blk in nc.m.functions[0].blocks:
            for inst in blk.instructions:
                names = set(inst.sync_dependency_names()) | set(
                    inst.nosync_dependency_names()
                )
                if names & removed.keys():
                    inst.remap_dependency_names(
                        {k: v for k, v in removed.items() if k in names}
                    )
        for k in removed:
            nc.inst_map.pop(k, None)
    return len(removed)


def _build_nc():
    # Bacc (not raw Bass): its compile() pass splits multi-wait instructions
    # into event semaphores — walrus allows only one sync wait per instruction.
    nc = bacc.Bacc()
    q = nc.dram_tensor("q", [NPAIR * 128, QCOLS], F8, kind="ExternalInput")
    pT = nc.dram_tensor("pT", [128, KT * NWAY], F8, kind="ExternalInput")
    negpn2 = nc.dram_tensor("negpn2", [128, 1], F32, kind="ExternalInput")
    bsel2 = nc.dram_tensor("bsel2", [128, 2 * NPAIR - 1], BF16, kind="ExternalInput")
    rsum = nc.dram_tensor("rsum", [128, NPAIR], F32, kind="ExternalOutput")
    sexp = nc.dram_tensor("sexp", [NPAIR, PAIRCOLS], F32, kind="ExternalOutput")
    sexpb = nc.dram_tensor("sexpb", [2, PAIRCOLS], F32, kind="ExternalOutput")

    with tile.TileContext(nc) as tc:
        with (
            tc.tile_pool(name="const", bufs=1) as cpool,
            tc.tile_pool(name="qin", bufs=4) as qpool,
            tc.tile_pool(name="qtail", bufs=1) as tpool,
            tc.tile_pool(name="acc", bufs=1) as apool,
            tc.tile_pool(name="gps", bufs=2 * UPG, space="PSUM") as gpool,
            tc.tile_pool(name="sps", bufs=1, space="PSUM") as spool,
        ):
            # First query-group DMA is issued before the const DMAs (further
            # below) so the big HBM stream starts as early as possible; the
            # constants land while the first group is still in flight.
            gt0 = qpool.tile([128, 2 * UPG * QCOLS], F8, name="gt", tag="gt")
            nc.sync.dma_start(
                gt0[:, 0 : 2 * UPG * QCOLS].rearrange(
                    "p (j c) -> p j c", c=QCOLS
                ),
                q[0 : 2 * UPG * 128, :].rearrange("(j p) c -> p j c", p=128),
            )

            p_sb = cpool.tile([128, KT * NWAY], F8)
            nc.sync.dma_start(p_sb[:], pT[:])
            npn_sb = cpool.tile([128, 1], F32)
            nc.sync.dma_start(npn_sb[:], negpn2[:])
            bsel_sb = cpool.tile([128, 2 * NPAIR - 1], BF16)
            nc.sync.dma_start(bsel_sb[:], bsel2[:])

            r_sb = apool.tile([128, NPAIR], F32)
            s_sb = apool.tile([NPAIR, PAIRCOLS], F32)
            sb_sb = apool.tile([2, PAIRCOLS], F32)
            e_all = apool.tile([128, NU * PAIRCOLS], BF16)
            s_ps = spool.tile([NPAIR, PAIRCOLS], F32, name="sps", tag="sps")
            s_psb = spool.tile([2, PAIRCOLS], F32, name="spsb", tag="spsb")

            # ACT warmup: absorb the npn DMA wait, the const-AP init wait and
            # the exp table load outside the hot loop.  Exp is the ONLY table
            # function used on device (the final log runs on host, on the
            # shipped colsum tile) so no 1.3us table swap ever happens.
            warm_a = cpool.tile([128, 1], F32)
            warm_b = cpool.tile([128, 1], F32)
            nc.scalar.copy(warm_a[:], npn_sb[:])
            nc.scalar.activation(
                warm_b[:], warm_a[:], mybir.ActivationFunctionType.Exp
            )

            def sel_matmul(u):
                # s_ps[2u, :]   += colsum over partitions 0..63  of e(u)
                # s_ps[2u+1, :] += colsum over partitions 64..127 of e(u)
                # Units 0..NU-2 accumulate in s_ps (shipped while the tail
                # unit computes); the tail unit gets its own tiny 2-row bank
                # so the end-of-kernel chain only ships 3 KB.
                nc.tensor.matmul(
                    s_ps[:],
                    bsel_sb[:, NPAIR - 1 - 2 * u : 2 * NPAIR - 1 - 2 * u],
                    e_all[:, u * PAIRCOLS : (u + 1) * PAIRCOLS],
                    start=(u == 0),
                    stop=(u == NU - 2),
                    skip_group_check=True,
                )

            def unit_post(u, gps_u):
                nc.scalar.activation(
                    e_all[:, u * PAIRCOLS : (u + 1) * PAIRCOLS],
                    gps_u[:],
                    mybir.ActivationFunctionType.Exp,
                    bias=npn_sb[:],
                    scale=2.0,
                )
                nc.vector.reduce_sum(
                    r_sb[:, 2 * u : 2 * u + 1],
                    gps_u[:, 0:F2],
                    axis=mybir.AxisListType.X,
                )
                nc.vector.reduce_sum(
                    r_sb[:, 2 * u + 1 : 2 * u + 2],
                    gps_u[:, F2:PAIRCOLS],
                    axis=mybir.AxisListType.X,
                )

            groups = [
                list(range(g, g + UPG)) for g in range(0, NU - 1, UPG)
            ]
            for gi, units in enumerate(groups):
                gp = 2 * len(units)          # pairs in this group
                p0 = 2 * units[0]            # first pair index
                # One big DMA per group (up to 2.4 MB): removes the per-pair
                # tile slot pressure that made the scheduler serialize units
                # (defeating weight reuse) and amortizes DMA fixed costs.
                if gi == 0:
                    gt = gt0
                else:
                    gt = qpool.tile(
                        [128, 2 * UPG * QCOLS], F8, name="gt", tag="gt"
                    )
                    nc.sync.dma_start(
                        gt[:, 0 : gp * QCOLS].rearrange(
                            "p (j c) -> p j c", c=QCOLS
                        ),
                        q[p0 * 128 : (p0 + gp) * 128, :].rearrange(
                            "(j p) c -> p j c", p=128
                        ),
                    )
                gps = {
                    u: gpool.tile([128, PAIRCOLS], F32, name="gps", tag="gps")
                    for u in units
                }
                for k in range(KT):
                    wk = p_sb[:, k * NWAY : (k + 1) * NWAY]
                    for jloc, u in enumerate(units):
                        ca = 2 * jloc * QCOLS + k * PAIRCOLS
                        cb = (2 * jloc + 1) * QCOLS + k * PAIRCOLS
                        nc.tensor.matmul(
                            gps[u][0:NWAY, :],
                            wk,
                            gt[:, ca : ca + PAIRCOLS],
                            tile_position=(0, 0),
                            start=(k == 0),
                            stop=(k == KT - 1),
                            skip_group_check=True,
                        )
                        nc.tensor.matmul(
                            gps[u][NWAY:128, :],
                            wk,
                            gt[:, cb : cb + PAIRCOLS],
                            tile_position=(0, NWAY),
                            start=(k == 0),
                            stop=(k == KT - 1),
                            skip_group_check=True,
                        )
                # Selector matmuls lag one group so the PE never stalls on
                # the ACT exp (exp(g-1) ran during this group's matmuls).
                if gi > 0:
                    for u in groups[gi - 1]:
                        sel_matmul(u)
                for u in units:
                    unit_post(u, gps[u])

            # ---- serial tail: the last unit (pairs 30, 31) ----
            # Streamed as pair-a whole (6272 B descriptor runs), then pair-b
            # in two half-k chunks (3136 B runs) — descriptor runs stay fat
            # so the tail rides the dense HBM stream instead of trailing it
            # with slow sub-KB packets.  Only pair-b's last 4 k-chunks of
            # matmul, the exp, one 2-row selector matmul and a 3 KB ship-out
            # remain after the stream ends.
            ut = NU - 1
            p0 = 2 * ut
            half = (KT // 2) * PAIRCOLS
            ta = tpool.tile([128, QCOLS], F8, name="ta", tag="ta")
            nc.sync.dma_start(
                ta[:], q[p0 * 128 : (p0 + 1) * 128, :]
            )
            tb = []
            for hi in range(2):
                h = tpool.tile([128, half], F8, name=f"tb{hi}", tag=f"tb{hi}")
                nc.sync.dma_start(
                    h[:],
                    q[(p0 + 1) * 128 : (p0 + 2) * 128, hi * half : (hi + 1) * half],
                )
                tb.append(h)
            # Bulk of the label-rowsum output can ship while the tail
            # finishes (columns 0..2*NU-3 are final before the last unit).
            nc.sync.dma_start(
                rsum[:, 0 : 2 * (NU - 1)], r_sb[:, 0 : 2 * (NU - 1)]
            )
            gps_t = gpool.tile([128, PAIRCOLS], F32, name="gps", tag="gps")
            for k in range(KT):
                nc.tensor.matmul(
                    gps_t[0:NWAY, :],
                    p_sb[:, k * NWAY : (k + 1) * NWAY],
                    ta[:, k * PAIRCOLS : (k + 1) * PAIRCOLS],
                    tile_position=(0, 0),
                    start=(k == 0),
                    stop=(k == KT - 1),
                    skip_group_check=True,
                )
            # Previous group's selector matmuls + the early s ship-out run
            # on the PE/ACT while pair-b's data lands.
            for u in groups[-1]:
                sel_matmul(u)
            nc.scalar.copy(s_sb[:], s_ps[:])
            nc.sync.dma_start(sexp[:], s_sb[:])
            for k in range(KT):
                src = tb[0] if k < KT // 2 else tb[1]
                cc = (k % (KT // 2)) * PAIRCOLS
                nc.tensor.matmul(
                    gps_t[NWAY:128, :],
                    p_sb[:, k * NWAY : (k + 1) * NWAY],
                    src[:, cc : cc + PAIRCOLS],
                    tile_position=(0, NWAY),
                    start=(k == 0),
                    stop=(k == KT - 1),
                    skip_group_check=True,
                )
            unit_post(ut, gps_t)
            # Tail unit's colsums land in their own tiny 2-row PSUM bank.
            nc.tensor.matmul(
                s_psb[:],
                bsel_sb[:, NPAIR - 1 : NPAIR + 1],
                e_all[:, ut * PAIRCOLS : (ut + 1) * PAIRCOLS],
                start=True,
                stop=True,
                skip_group_check=True,
            )
            nc.scalar.copy(sb_sb[:], s_psb[:])
            nc.sync.dma_start(
                rsum[:, 2 * (NU - 1) : 2 * NU], r_sb[:, 2 * (NU - 1) : 2 * NU]
            )
            nc.sync.dma_start(sexpb[:], sb_sb[:])

    n = _dedup_ldweights(nc)
    if n < 64:
        print(f"[kernel] warning: ldweights dedup removed only {n}", flush=True)
    nc.compile()
    return nc


def _get_nc():
    if "nc" not in _CACHE:
        _CACHE["nc"] = _build_nc()
    return _CACHE["nc"]


def _pack_core_q(qc32):
    # [64, C, F2] -> [pair, p, k, i, f] -> [NPAIR*128, QCOLS] fp16
    qc = qc32.reshape(NPAIR, 2, KT, 128, F2).transpose(0, 3, 2, 1, 4)
    return np.ascontiguousarray(qc, dtype=F8_NP).reshape(NPAIR * 128, QCOLS)


def _prepare(query_features, labels, prototypes, indices):
    """Returns (in_maps, labels_i64, pn32)."""
    qf = np.asarray(query_features, dtype=np.float32).reshape(B, C, F2)
    labels = np.asarray(labels).astype(np.int64)
    protos = np.asarray(prototypes, dtype=np.float32)
    idx = np.asarray(indices).astype(np.int64)

    pg = protos[idx]                                     # [64, C] fp32
    pn32 = np.sum(pg.astype(np.float64) ** 2, axis=1).astype(np.float32)
    negpn2_np = np.ascontiguousarray(
        np.concatenate([-pn32, -pn32]).reshape(128, 1)
    )
    pT_pack = np.ascontiguousarray(
        pg.T.reshape(KT, 128, NWAY).transpose(1, 0, 2), dtype=F8_NP
    ).reshape(128, KT * NWAY)
    bsel2_np = np.zeros((128, 2 * NPAIR - 1), dtype=BF16_NP)
    bsel2_np[0:NWAY, NPAIR - 1] = 1
    bsel2_np[NWAY:128, NPAIR] = 1

    in_maps = [
        {
            "q": _pack_core_q(qf[c * BPC : (c + 1) * BPC]),
            "pT": pT_pack,
            "negpn2": negpn2_np,
            "bsel2": bsel2_np,
        }
        for c in range(NCORES)
    ]
    return in_maps, labels, pn32


def kernel(query_features, labels, prototypes, indices, n_way):
    import time as _time

    t0 = _time.time()
    nc = _get_nc()
    t1 = _time.time()
    in_maps, labels, pn32 = _prepare(query_features, labels, prototypes, indices)
    t2 = _time.time()
    results = run_bass_kernel_spmd(nc, in_maps, list(range(NCORES))).results
    t3 = _time.time()
    print(
        f"[kernel] build={t1 - t0:.1f}s pack={t2 - t1:.1f}s run={t3 - t2:.1f}s",
        flush=True,
    )

    # Host-side finish: r2[128, 32] holds per-image rowsums of G; image
    # local index l lives at row block 64*(l%4>=2)+class, column 2*(l//4)+(l%2).
    pn64 = pn32.astype(np.float64)
    larr = np.arange(BPC)
    rows0 = 64 * ((larr % 4) >= 2)
    cols = 2 * (larr // 4) + (larr % 2)
    total_lse = 0.0
    label_term = 0.0
    for c in range(NCORES):
        total_lse += float(
            np.log(results[c]["sexp"][0 : 2 * (NU - 1)].astype(np.float64)).sum()
        ) + float(np.log(results[c]["sexpb"].astype(np.float64)).sum())
        r2 = results[c]["rsum"].astype(np.float64)       # [128, 32]
        lab = labels[c * BPC : (c + 1) * BPC]
        label_term += float(
            np.sum(2.0 * r2[rows0 + lab, cols] - F2 * pn64[lab])
        )
    loss = (total_lse - label_term) / (B * F2)
    return np.asarray(loss, dtype=np.float32)

